# revision 13
# baseline (speedup 1.0000x reference)
"""Trainium2 Bass kernel for nn_CompactBilinearPoolingTSP.

Count-sketch + FFT circular convolution collapses (Parseval) into dense
half-spectrum DFT matmuls: F[r,k] = sum_c X[r,c] E[c,k] with E a host
constant, Phi = F1*F2, ip[r] = (1/D) sum_k gamma_k Re(Phi conj(F1y F2y)).
The y-side (sensor branch) is rank-1 in s, so its spectra reduce to
per-b vectors; appended t rows + a ones row ride the same matmuls.

v2 layout: xt is fully host-built ([128, kt, rows] f16, t rows + ones
appended); the main loop computes the 4 DFT planes per 128-freq tile
into PSUM and forms Phi = F1*F2 directly from PSUM (R chain on DVE,
I chain on GpSimd) — no intermediate SBUF spectra.  Pass 2 (contraction
of Phi against the 3 per-b y-vectors over k) is interleaved per 8-tile
batch.  The Nyquist bin k=4096 is handled exactly by a 2-column matmul.
Device emits T[12,585] + Nyquist spectra; signed-sqrt/normalize/W_out
tail runs on host.  Sharding: pure data parallel, 4 batches/core.
"""

import numpy as np

try:
    import concourse.bass  # noqa: F401
except ImportError:  # pragma: no cover
    import sys
    for _p in ("/opt/trn_rl_repo", "/root/.axon_site/_ro/trn_rl_repo"):
        if _p not in sys.path:
            sys.path.append(_p)

_PROGRAM = None

B, S, C, D, SN = 32, 145, 768, 8192, 64
NCORES = 8
BC = B // NCORES          # batches per core = 4
NRX = BC * S              # x rows per core = 580
NR = NRX + BC + 1         # + t rows + ones row = 585
KF = D // 2 + 1           # 4097 distinct freqs
NFT = 32                  # freq tiles of 128 -> 4096; k=4096 handled exactly
KP = NFT * 128
KT = C // 128             # 6 contraction tiles
CH = [(0, 293), (293, 292)]  # row chunks for matmul N (fit one PSUM bank)
NWARM = 16                # PE warm-up matmuls during DMA head


def _host_constants(h1, h2, s1, s2):
    """E matrices, gamma, V3, Nyquist columns — from hash/sign vectors."""
    h1 = h1.astype(np.int64); h2 = h2.astype(np.int64)
    s1f = s1.astype(np.float64); s2f = s2.astype(np.float64)
    k = np.arange(KP)
    ang1 = (-2.0 * np.pi / D) * (h1[:, None] * k[None, :])
    ang2 = (-2.0 * np.pi / D) * (h2[:, None] * k[None, :])
    E1 = s1f[:, None] * np.exp(1j * ang1)
    E2 = s2f[:, None] * np.exp(1j * ang2)
    # planes: 0=E1r 1=E1i 2=E2r 3=E2i ; layout [NFT, 128k, KT, plane, 128f]
    E = np.stack([E1.real, E1.imag, E2.real, E2.imag], axis=0)  # [4, C, KP]
    E = E.reshape(4, KT, 128, NFT, 128)                          # [p, kt, k, ft, f]
    E = E.transpose(3, 2, 1, 0, 4)                               # [ft, k, kt, p, f]
    E = np.ascontiguousarray(E, dtype=np.float16)

    gamma = np.full(KP, 2.0)
    gamma[0] = 1.0
    gamma_sb = gamma.reshape(NFT, 128).T.astype(np.float32)      # [128, NFT]

    # V3 = gamma * (Q1*Q2) (ones-row spectra product), exact on host
    Q1 = np.ones(C) @ E1
    Q2 = np.ones(C) @ E2
    W3 = Q1 * Q2
    v3 = np.stack([(gamma * W3.real), (gamma * W3.imag)], axis=-1)  # [KP, 2]
    v3_sb = v3.reshape(NFT, 128, 2).transpose(1, 0, 2)              # [128, NFT, 2]
    v3_sb = np.ascontiguousarray(v3_sb, dtype=np.float16)

    # Nyquist k=4096 columns: e[c] = s[c] * (-1)^{h[c]}, per hash
    en = np.stack([s1f * np.where(h1 % 2 == 0, 1.0, -1.0),
                   s2f * np.where(h2 % 2 == 0, 1.0, -1.0)], axis=-1)  # [C, 2]
    en_sb = np.ascontiguousarray(
        en.reshape(KT, 128, 2).transpose(1, 0, 2), dtype=np.float16)  # [128,KT,2]
    return E, gamma_sb, v3_sb, en_sb


def _host_inputs_for_core(core, inputs, E, gamma_sb, v3_sb, en_sb):
    """Per-core in_map (numpy) keyed by dram tensor names."""
    img = np.asarray(inputs["image_embeds"], np.float32)
    sensor = np.asarray(inputs["sensor"], np.float32)
    b0 = core * BC
    rows = np.empty((NR, C), np.float32)
    rows[:NRX] = (img[b0:b0 + BC]
                  + np.asarray(inputs["tok_emb"], np.float32)[1][None, None, :]
                  ).reshape(NRX, C)
    # sensor branch t rows: [BC, C]
    t = (sensor[b0:b0 + BC, 0, :] @ np.asarray(inputs["W_sensor"], np.float32).T
         + np.asarray(inputs["b_sensor"], np.float32)[None, :])
    rows[NRX:NRX + BC] = t
    rows[NR - 1] = 1.0
    # xt layout: [128 part, KT, NR] f16  (channel c = kt*128 + p)
    xtc = np.ascontiguousarray(
        rows.T.reshape(KT, 128, NR).transpose(1, 0, 2), dtype=np.float16)
    return {
        "xtc": xtc,
        "Econst": E,
        "gammac": gamma_sb,
        "v3c": v3_sb,
        "enc": en_sb,
        "warmc": np.zeros((128, 512), np.float16),
    }


def _build_program():
    import concourse.tile as tile
    from concourse import bacc, mybir

    f16 = mybir.dt.float16
    f32 = mybir.dt.float32
    OP = mybir.AluOpType

    nc = bacc.Bacc("TRN2", target_bir_lowering=False, debug=False,
                   num_devices=NCORES)

    xtc = nc.dram_tensor("xtc", [128, KT, NR], f16, kind="ExternalInput")
    Ec = nc.dram_tensor("Econst", [NFT, 128, KT, 4, 128], f16,
                        kind="ExternalInput")
    gammac = nc.dram_tensor("gammac", [128, NFT], f32, kind="ExternalInput")
    v3c = nc.dram_tensor("v3c", [128, NFT, 2], f16, kind="ExternalInput")
    enc = nc.dram_tensor("enc", [128, KT, 2], f16, kind="ExternalInput")
    warmc = nc.dram_tensor("warmc", [128, 512], f16, kind="ExternalInput")
    tsb_d = nc.dram_tensor("tsb_out", [12, NR], f32, kind="ExternalOutput")
    nyq_d = nc.dram_tensor("nyq_out", [2, NR], f32, kind="ExternalOutput")

    with tile.TileContext(nc) as tc:
        with (
            tc.tile_pool(name="const", bufs=1) as cp,
            tc.tile_pool(name="estream", bufs=2) as ep,
            tc.tile_pool(name="vtmp", bufs=2) as vp,
            tc.tile_pool(name="ptmp", bufs=2) as qp,
            tc.tile_pool(name="phip", bufs=1) as pp,
        ):
            # ---- persistent tiles ----
            xt = cp.tile([128, KT, NR], f16)          # rows^T (c on partitions)
            phiR = pp.tile([128, NFT, NR], f16)
            phiI = pp.tile([128, NFT, NR], f16)
            fy = cp.tile([128, NFT, 4, 5], f16)       # spectra of t rows + ones
            vt = cp.tile([128, NFT, 2, BC, 3], f16)   # lhsT for pass 2
            gam = cp.tile([128, NFT], f32)
            v3s = cp.tile([128, NFT, 2], f16)
            en = cp.tile([128, KT, 2], f16)
            tsb = cp.tile([12, NR], f32)
            nyqs = cp.tile([2, NR], f32)
            wt = cp.tile([128, 512], f16)
            sy = nc.sync
            # warm tile first, then xt + consts (scalar queue);
            # E stream rides the sync queue.
            nc.scalar.dma_start(wt[:], warmc.ap())
            nc.scalar.dma_start(xt[:], xtc.ap())
            nc.scalar.dma_start(gam[:], gammac.ap())
            nc.scalar.dma_start(v3s[:], v3c.ap())
            nc.scalar.dma_start(en[:], enc.ap())

            VGROUPS = {7: (0, 8), 15: (8, 16), 23: (16, 24), 31: (24, NFT)}

            def build_v_group(g0, g1):
                ng = g1 - g0
                sl = slice(g0, g1)
                P1r = fy[:, sl, 0, 0:BC]; P1i = fy[:, sl, 1, 0:BC]
                P2r = fy[:, sl, 2, 0:BC]; P2i = fy[:, sl, 3, 0:BC]
                shp = (128, ng, BC)
                Q1r = fy[:, sl, 0, 4:5].to_broadcast(shp)
                Q1i = fy[:, sl, 1, 4:5].to_broadcast(shp)
                Q2r = fy[:, sl, 2, 4:5].to_broadcast(shp)
                Q2i = fy[:, sl, 3, 4:5].to_broadcast(shp)
                gb = gam[:, sl, None].to_broadcast(shp)
                va = vp.tile([128, 8, BC], f32, tag="va", name="va")[:, :ng, :]
                vb = vp.tile([128, 8, BC], f32, tag="vb", name="vb")[:, :ng, :]
                vc = vp.tile([128, 8, BC], f32, tag="vc", name="vc")[:, :ng, :]
                TT = nc.vector.tensor_tensor
                TT(va[:], P1r, P2r, OP.mult)
                TT(vb[:], P1i, P2i, OP.mult)
                TT(vc[:], va[:], vb[:], OP.subtract)
                TT(vt[:, sl, 0, :, 0], vc[:], gb, OP.mult)
                TT(va[:], P1r, P2i, OP.mult)
                TT(vb[:], P1i, P2r, OP.mult)
                TT(vc[:], va[:], vb[:], OP.add)
                TT(vt[:, sl, 1, :, 0], vc[:], gb, OP.mult)
                TT(va[:], P1r, Q2r, OP.mult)
                TT(vb[:], P1i, Q2i, OP.mult)
                TT(va[:], va[:], vb[:], OP.subtract)
                TT(vb[:], P2r, Q1r, OP.mult)
                TT(vc[:], P2i, Q1i, OP.mult)
                TT(vb[:], vb[:], vc[:], OP.subtract)
                TT(va[:], va[:], vb[:], OP.add)
                TT(vt[:, sl, 0, :, 1], va[:], gb, OP.mult)
                TT(va[:], P1r, Q2i, OP.mult)
                TT(vb[:], P1i, Q2r, OP.mult)
                TT(va[:], va[:], vb[:], OP.add)
                TT(vb[:], P2r, Q1i, OP.mult)
                TT(vc[:], P2i, Q1r, OP.mult)
                TT(vb[:], vb[:], vc[:], OP.add)
                TT(va[:], va[:], vb[:], OP.add)
                TT(vt[:, sl, 1, :, 1], va[:], gb, OP.mult)
                nc.vector.tensor_copy(
                    vt[:, sl, :, :, 2],
                    v3s[:, sl, :, None].to_broadcast((128, ng, 2, BC)))

            with (
                tc.tile_pool(name="mps", bufs=7, space="PSUM") as mps,
                tc.tile_pool(name="p2ps", bufs=1, space="PSUM") as p2,
            ):
                # ---- PE warm-up (HAM ramp) while xt/E stream in ----
                for w in range(NWARM):
                    wps = mps.tile([128, 293], f32, tag="mm", name=f"warm{w}")
                    nc.tensor.matmul(wps[:, :293], wt[:, 0:128], wt[:, 0:293],
                                     start=True, stop=True)
                # ---- Nyquist bin k=4096: F[4096] = sum_c x s (-1)^h ----
                nyp = [mps.tile([128, 293], f32, tag="mm", name=f"nyp{c}")
                       for c in range(2)]
                for ci, (c0, nn) in enumerate(CH):
                    for kt in range(KT):
                        nc.tensor.matmul(
                            nyp[ci][:2, :nn], en[:, kt, :],
                            xt[:, kt, c0:c0 + nn],
                            start=(kt == 0), stop=(kt == KT - 1))
                for ci, (c0, nn) in enumerate(CH):
                    nc.scalar.copy(nyqs[:, c0:c0 + nn], nyp[ci][:2, :nn])
                sy.dma_start(nyq_d.ap(), nyqs[:])

                tps = p2.tile([12, 512], f32)

                def pass2_batch(g0, g1):
                    build_v_group(g0, g1)
                    for c0, nn in ((0, 512), (512, 73)):
                        for g in range(g0, g1):
                            nc.tensor.matmul(
                                tps[:, :nn],
                                vt[:, g, 0, :, :].rearrange("p b j -> p (b j)"),
                                phiR[:, g, c0:c0 + nn],
                                start=(g == g0), stop=False,
                                skip_group_check=True)
                            nc.tensor.matmul(
                                tps[:, :nn],
                                vt[:, g, 1, :, :].rearrange("p b j -> p (b j)"),
                                phiI[:, g, c0:c0 + nn],
                                start=False, stop=(g == g1 - 1),
                                skip_group_check=True)
                        if g0 == 0:
                            nc.vector.tensor_copy(tsb[:, c0:c0 + nn],
                                                  tps[:, :nn])
                        else:
                            nc.vector.tensor_tensor(
                                tsb[:, c0:c0 + nn], tsb[:, c0:c0 + nn],
                                tps[:, :nn], OP.add)

                # ---- main loop over 32 frequency tiles ----
                # plane order (2,0,1,3): the 8th PSUM alloc (bufs=7) aliases
                # plane 2's slot, whose product consumers finish earliest.
                o0 = NRX - CH[1][0]   # y-rows offset within chunk 1
                for ft in range(NFT):
                    et = ep.tile([128, KT, 4, 128], f16, tag="et")
                    sy.dma_start(et[:], Ec.ap()[ft])
                    ps = {}

                    def plane(p):
                        for ci, (c0, nn) in enumerate(CH):
                            ps[(p, ci)] = mps.tile([128, 293], f32, tag="mm",
                                                   name=f"mm{p}{ci}")
                        for kt in range(KT):
                            st = (kt == 0); sp = (kt == KT - 1)
                            for ci, (c0, nn) in enumerate(CH):
                                nc.tensor.matmul(
                                    ps[(p, ci)][:, :nn], et[:, kt, p, :],
                                    xt[:, kt, c0:c0 + nn], start=st, stop=sp)
                        nc.scalar.copy(fy[:, ft, p, :], ps[(p, 1)][:, o0:o0 + 5])

                    # GPSIMD has no PSUM port; DVE TT allows one PSUM operand.
                    # scalar: copy planes 2,3 to SBUF; DVE: the 4 products;
                    # GpSimd: the SBUF-only combines.
                    TTv = nc.vector.tensor_tensor
                    TTg = nc.gpsimd.tensor_tensor
                    plane(2)
                    s2 = [qp.tile([128, 293], f32, tag=f"s2{ci}", name=f"s2{ci}")
                          for ci in range(2)]
                    for ci, (c0, nn) in enumerate(CH):
                        nc.scalar.copy(s2[ci][:, :nn], ps[(2, ci)][:, :nn])
                    plane(0)
                    ta = [qp.tile([128, 293], f32, tag=f"ta{ci}", name=f"ta{ci}")
                          for ci in range(2)]
                    for ci, (c0, nn) in enumerate(CH):      # ta = F1r*F2r
                        TTv(ta[ci][:, :nn], ps[(0, ci)][:, :nn],
                            s2[ci][:, :nn], OP.mult)
                    plane(1)
                    th = [qp.tile([128, 293], f32, tag=f"th{ci}", name=f"th{ci}")
                          for ci in range(2)]
                    for ci, (c0, nn) in enumerate(CH):      # th = F1i*F2r
                        TTv(th[ci][:, :nn], ps[(1, ci)][:, :nn],
                            s2[ci][:, :nn], OP.mult)
                    plane(3)
                    s3 = [qp.tile([128, 293], f32, tag=f"s3{ci}", name=f"s3{ci}")
                          for ci in range(2)]
                    for ci, (c0, nn) in enumerate(CH):
                        nc.scalar.copy(s3[ci][:, :nn], ps[(3, ci)][:, :nn])
                    for ci, (c0, nn) in enumerate(CH):
                        sl = slice(c0, c0 + nn)
                        tb = qp.tile([128, 293], f32, tag="tb", name="tb")
                        TTv(tb[:, :nn], ps[(1, ci)][:, :nn],
                            s3[ci][:, :nn], OP.mult)        # tb = F1i*F2i
                        TTg(phiR[:, ft, sl], ta[ci][:, :nn], tb[:, :nn],
                            OP.subtract)
                        tg = qp.tile([128, 293], f32, tag="tg", name="tg")
                        TTv(tg[:, :nn], ps[(0, ci)][:, :nn],
                            s3[ci][:, :nn], OP.mult)        # tg = F1r*F2i
                        TTg(phiI[:, ft, sl], tg[:, :nn], th[ci][:, :nn],
                            OP.add)
                    # pass-2 batches trail by one ft so vgroup (DVE) and the
                    # trailing phi combines overlap this ft's matmul window
                    if ft in (8, 16, 24):
                        pass2_batch(ft - 8, ft)
                pass2_batch(24, NFT - 1)
                pass2_batch(NFT - 1, NFT)

            sy.dma_start(tsb_d.ap(), tsb[:])

    nc.compile()
    return nc


def _host_tail(inputs, results):
    """Combine per-core T/nyq into the final [B,1] output on host."""
    w2 = np.asarray(inputs["W_s2"], np.float64)[:, 0]            # [S]
    beta = np.asarray(inputs["b_s2"], np.float64)                # [S]
    wv = np.stack([w2 * w2, w2 * beta, beta * beta], 0) / D      # [3, S]
    W_out = np.asarray(inputs["W_out"], np.float64)              # [1, S]
    b_out = np.asarray(inputs["b_out"], np.float64)              # [1]
    out = np.empty((B, 1), np.float64)
    for core in range(NCORES):
        T = np.asarray(results[core]["tsb_out"], np.float64).reshape(12, NR)
        ny = np.asarray(results[core]["nyq_out"], np.float64).reshape(2, NR)
        F1n, F2n = ny[0], ny[1]
        Q1n, Q2n = F1n[NR - 1], F2n[NR - 1]
        for b in range(BC):
            Tb = T[b * 3:(b + 1) * 3, b * S:(b + 1) * S].copy()  # [3, S]
            pxn = F1n[b * S:(b + 1) * S] * F2n[b * S:(b + 1) * S]
            T1n, T2n = F1n[NRX + b], F2n[NRX + b]
            Tb[0] += pxn * (T1n * T2n)
            Tb[1] += pxn * (T1n * Q2n + T2n * Q1n)
            Tb[2] += pxn * (Q1n * Q2n)
            ip = wv[0] * Tb[0] + wv[1] * Tb[1] + wv[2] * Tb[2]   # [S]
            bp = np.sign(ip) * np.sqrt(np.abs(ip) + 1e-5)
            nrm = max(np.linalg.norm(bp), 1e-12)
            bp = bp / nrm
            out[core * BC + b, 0] = bp @ W_out[0] + b_out[0]
    return out.astype(np.float32)


def kernel(**inputs) -> np.ndarray:
    global _PROGRAM
    if _PROGRAM is None:
        _PROGRAM = _build_program()
    nc = _PROGRAM

    consts = _host_constants(
        inputs["h1"], inputs["h2"], inputs["s1"], inputs["s2"])
    in_maps = [_host_inputs_for_core(c, inputs, *consts)
               for c in range(NCORES)]

    from concourse.bass_utils import run_bass_kernel_spmd
    res = run_bass_kernel_spmd(nc, in_maps, list(range(NCORES)))
    return _host_tail(inputs, res.results)


# revision 18
# speedup vs baseline: 1.0763x; 1.0763x over previous
"""Trainium2 Bass kernel for nn_CompactBilinearPoolingTSP.

Count-sketch + FFT circular convolution collapses (Parseval) into dense
half-spectrum DFT matmuls: F[r,k] = sum_c X[r,c] E[c,k] with E a host
constant, Phi = F1*F2, ip[r] = (1/D) sum_k gamma_k Re(Phi conj(F1y F2y)).
The y-side (sensor branch) is rank-1 in s, so its spectra reduce to
per-b vectors; appended t rows + a ones row ride the same matmuls.

v2 layout: xt is fully host-built ([128, kt, rows] f16, t rows + ones
appended); the main loop computes the 4 DFT planes per 128-freq tile
into PSUM and forms Phi = F1*F2 directly from PSUM (R chain on DVE,
I chain on GpSimd) — no intermediate SBUF spectra.  Pass 2 (contraction
of Phi against the 3 per-b y-vectors over k) is interleaved per 8-tile
batch.  The Nyquist bin k=4096 is handled exactly by a 2-column matmul.
Device emits T[12,585] + Nyquist spectra; signed-sqrt/normalize/W_out
tail runs on host.  Sharding: pure data parallel, 4 batches/core.
"""

import numpy as np

try:
    import concourse.bass  # noqa: F401
except ImportError:  # pragma: no cover
    import sys
    for _p in ("/opt/trn_rl_repo", "/root/.axon_site/_ro/trn_rl_repo"):
        if _p not in sys.path:
            sys.path.append(_p)

_PROGRAM = None

B, S, C, D, SN = 32, 145, 768, 8192, 64
NCORES = 8
BC = B // NCORES          # batches per core = 4
NRX = BC * S              # x rows per core = 580
NR = NRX + BC + 1         # + t rows + ones row = 585
KF = D // 2 + 1           # 4097 distinct freqs
NFT = 32                  # freq tiles of 128 -> 4096; k=4096 handled exactly
KP = NFT * 128
KT = C // 128             # 6 contraction tiles
CH = [(0, 293), (293, 292)]  # row chunks for matmul N (fit one PSUM bank)
NWARM = 16                # PE warm-up matmuls during DMA head


def _host_constants(h1, h2, s1, s2):
    """E matrices, gamma, V3, Nyquist columns — from hash/sign vectors."""
    h1 = h1.astype(np.int64); h2 = h2.astype(np.int64)
    s1f = s1.astype(np.float64); s2f = s2.astype(np.float64)
    k = np.arange(KP)
    ang1 = (-2.0 * np.pi / D) * (h1[:, None] * k[None, :])
    ang2 = (-2.0 * np.pi / D) * (h2[:, None] * k[None, :])
    E1 = s1f[:, None] * np.exp(1j * ang1)
    E2 = s2f[:, None] * np.exp(1j * ang2)
    # planes: 0=E1r 1=E1i 2=E2r 3=E2i ; layout [NFT, 128k, KT, plane, 128f]
    E = np.stack([E1.real, E1.imag, E2.real, E2.imag], axis=0)  # [4, C, KP]
    E = E.reshape(4, KT, 128, NFT, 128)                          # [p, kt, k, ft, f]
    E = E.transpose(3, 2, 1, 0, 4)                               # [ft, k, kt, p, f]
    E = np.ascontiguousarray(E, dtype=np.float16)

    gamma = np.full(KP, 2.0)
    gamma[0] = 1.0
    gamma_sb = gamma.reshape(NFT, 128).T.astype(np.float32)      # [128, NFT]

    # V3 = gamma * (Q1*Q2) (ones-row spectra product), exact on host
    Q1 = np.ones(C) @ E1
    Q2 = np.ones(C) @ E2
    W3 = Q1 * Q2
    v3 = np.stack([(gamma * W3.real), (gamma * W3.imag)], axis=-1)  # [KP, 2]
    v3_sb = v3.reshape(NFT, 128, 2).transpose(1, 0, 2)              # [128, NFT, 2]
    v3_sb = np.ascontiguousarray(v3_sb, dtype=np.float16)

    # Nyquist k=4096 columns: e[c] = s[c] * (-1)^{h[c]}, per hash
    en = np.stack([s1f * np.where(h1 % 2 == 0, 1.0, -1.0),
                   s2f * np.where(h2 % 2 == 0, 1.0, -1.0)], axis=-1)  # [C, 2]
    en_sb = np.ascontiguousarray(
        en.reshape(KT, 128, 2).transpose(1, 0, 2), dtype=np.float16)  # [128,KT,2]
    return E, gamma_sb, v3_sb, en_sb


def _host_inputs_for_core(core, inputs, E, gamma_sb, v3_sb, en_sb):
    """Per-core in_map (numpy) keyed by dram tensor names."""
    img = np.asarray(inputs["image_embeds"], np.float32)
    sensor = np.asarray(inputs["sensor"], np.float32)
    b0 = core * BC
    rows = np.empty((NR, C), np.float32)
    rows[:NRX] = (img[b0:b0 + BC]
                  + np.asarray(inputs["tok_emb"], np.float32)[1][None, None, :]
                  ).reshape(NRX, C)
    # sensor branch t rows: [BC, C]
    t = (sensor[b0:b0 + BC, 0, :] @ np.asarray(inputs["W_sensor"], np.float32).T
         + np.asarray(inputs["b_sensor"], np.float32)[None, :])
    rows[NRX:NRX + BC] = t
    rows[NR - 1] = 1.0
    # xt layout: [128 part, KT, NR] f16  (channel c = kt*128 + p)
    xtc = np.ascontiguousarray(
        rows.T.reshape(KT, 128, NR).transpose(1, 0, 2), dtype=np.float16)
    return {
        "xtc": xtc,
        "Econst": E,
        "gammac": gamma_sb,
        "v3c": v3_sb,
        "enc": en_sb,
        "warmc": np.zeros((128, 512), np.float16),
    }


def _build_program():
    import concourse.tile as tile
    from concourse import bacc, mybir

    f16 = mybir.dt.float16
    f32 = mybir.dt.float32
    OP = mybir.AluOpType

    nc = bacc.Bacc("TRN2", target_bir_lowering=False, debug=False,
                   num_devices=NCORES)

    xtc = nc.dram_tensor("xtc", [128, KT, NR], f16, kind="ExternalInput")
    Ec = nc.dram_tensor("Econst", [NFT, 128, KT, 4, 128], f16,
                        kind="ExternalInput")
    gammac = nc.dram_tensor("gammac", [128, NFT], f32, kind="ExternalInput")
    v3c = nc.dram_tensor("v3c", [128, NFT, 2], f16, kind="ExternalInput")
    enc = nc.dram_tensor("enc", [128, KT, 2], f16, kind="ExternalInput")
    warmc = nc.dram_tensor("warmc", [128, 512], f16, kind="ExternalInput")
    tsb_d = nc.dram_tensor("tsb_out", [12, NR], f32, kind="ExternalOutput")
    nyq_d = nc.dram_tensor("nyq_out", [2, NR], f32, kind="ExternalOutput")

    with tile.TileContext(nc) as tc:
        with (
            tc.tile_pool(name="const", bufs=1) as cp,
            tc.tile_pool(name="estream", bufs=2) as ep,
            tc.tile_pool(name="vtmp", bufs=2) as vp,
            tc.tile_pool(name="ptmp", bufs=2) as qp,
            tc.tile_pool(name="phip", bufs=1) as pp,
        ):
            # ---- persistent tiles ----
            xt = cp.tile([128, KT, NR], f16)          # rows^T (c on partitions)
            phiR = pp.tile([128, NFT, NR], f16)
            phiI = pp.tile([128, NFT, NR], f16)
            fy = cp.tile([128, NFT, 4, 5], f16)       # spectra of t rows + ones
            vtb = [cp.tile([128, 8, 2, BC, 3], f16, name=f"vtb{i}")
                   for i in range(2)]                 # lhsT for pass 2 (2 bufs)
            gam = cp.tile([128, NFT], f32)
            v3s = cp.tile([128, NFT, 2], f16)
            en = cp.tile([128, KT, 2], f16)
            tsb = cp.tile([12, NR], f32)
            nyqs = cp.tile([2, NR], f32)
            wt = cp.tile([128, 512], f16)
            sy = nc.sync
            # warm tile first, then xt + consts (scalar queue);
            # E stream rides the sync queue.
            nc.scalar.dma_start(wt[:], warmc.ap())
            nc.scalar.dma_start(xt[:], xtc.ap())
            nc.scalar.dma_start(gam[:], gammac.ap())
            nc.scalar.dma_start(v3s[:], v3c.ap())
            nc.scalar.dma_start(en[:], enc.ap())

            # pass-2 batches: vgroup built a full batch ahead (double-
            # buffered vt), MMs slotted mid-ft so nothing waits on DVE
            BATCHES = [(0, 8), (8, 16), (16, 24), (24, 31), (31, 32)]

            def build_v_group(b):
                g0, g1 = BATCHES[b]
                ng = g1 - g0
                sl = slice(g0, g1)
                vo = vtb[b % 2]
                P1r = fy[:, sl, 0, 0:BC]; P1i = fy[:, sl, 1, 0:BC]
                P2r = fy[:, sl, 2, 0:BC]; P2i = fy[:, sl, 3, 0:BC]
                shp = (128, ng, BC)
                Q1r = fy[:, sl, 0, 4:5].to_broadcast(shp)
                Q1i = fy[:, sl, 1, 4:5].to_broadcast(shp)
                Q2r = fy[:, sl, 2, 4:5].to_broadcast(shp)
                Q2i = fy[:, sl, 3, 4:5].to_broadcast(shp)
                gb = gam[:, sl, None].to_broadcast(shp)
                va = vp.tile([128, 8, BC], f32, tag="va", name="va")[:, :ng, :]
                vb = vp.tile([128, 8, BC], f32, tag="vb", name="vb")[:, :ng, :]
                vc = vp.tile([128, 8, BC], f32, tag="vc", name="vc")[:, :ng, :]
                TT = nc.vector.tensor_tensor
                TT(va[:], P1r, P2r, OP.mult)
                TT(vb[:], P1i, P2i, OP.mult)
                TT(vc[:], va[:], vb[:], OP.subtract)
                TT(vo[:, :ng, 0, :, 0], vc[:], gb, OP.mult)
                TT(va[:], P1r, P2i, OP.mult)
                TT(vb[:], P1i, P2r, OP.mult)
                TT(vc[:], va[:], vb[:], OP.add)
                TT(vo[:, :ng, 1, :, 0], vc[:], gb, OP.mult)
                TT(va[:], P1r, Q2r, OP.mult)
                TT(vb[:], P1i, Q2i, OP.mult)
                TT(va[:], va[:], vb[:], OP.subtract)
                TT(vb[:], P2r, Q1r, OP.mult)
                TT(vc[:], P2i, Q1i, OP.mult)
                TT(vb[:], vb[:], vc[:], OP.subtract)
                TT(va[:], va[:], vb[:], OP.add)
                TT(vo[:, :ng, 0, :, 1], va[:], gb, OP.mult)
                TT(va[:], P1r, Q2i, OP.mult)
                TT(vb[:], P1i, Q2r, OP.mult)
                TT(va[:], va[:], vb[:], OP.add)
                TT(vb[:], P2r, Q1i, OP.mult)
                TT(vc[:], P2i, Q1r, OP.mult)
                TT(vb[:], vb[:], vc[:], OP.add)
                TT(va[:], va[:], vb[:], OP.add)
                TT(vo[:, :ng, 1, :, 1], va[:], gb, OP.mult)
                nc.vector.tensor_copy(
                    vo[:, :ng, :, :, 2],
                    v3s[:, sl, :, None].to_broadcast((128, ng, 2, BC)))

            with (
                tc.tile_pool(name="mps", bufs=7, space="PSUM") as mps,
                tc.tile_pool(name="p2ps", bufs=1, space="PSUM") as p2,
            ):
                # ---- PE warm-up (HAM ramp) while xt/E stream in ----
                for w in range(NWARM):
                    wps = mps.tile([128, 293], f32, tag="mm", name=f"warm{w}")
                    nc.tensor.matmul(wps[:, :293], wt[:, 0:128], wt[:, 0:293],
                                     start=True, stop=True)
                # ---- Nyquist bin k=4096: F[4096] = sum_c x s (-1)^h ----
                nyp = [mps.tile([128, 293], f32, tag="mm", name=f"nyp{c}")
                       for c in range(2)]
                for ci, (c0, nn) in enumerate(CH):
                    for kt in range(KT):
                        nc.tensor.matmul(
                            nyp[ci][:2, :nn], en[:, kt, :],
                            xt[:, kt, c0:c0 + nn],
                            start=(kt == 0), stop=(kt == KT - 1))
                for ci, (c0, nn) in enumerate(CH):
                    nc.scalar.copy(nyqs[:, c0:c0 + nn], nyp[ci][:2, :nn])
                sy.dma_start(nyq_d.ap(), nyqs[:])

                # chunk0 accumulates at partitions 0:12, chunk1 at 32:44 —
                # disjoint byte ranges, so the two chunks never WAR-stall
                tps = p2.tile([44, 512], f32)
                P2CH = ((0, 0, 512), (32, 512, 73))

                def pass2_mms(b):
                    g0, g1 = BATCHES[b]
                    vo = vtb[b % 2]
                    for p0, c0, nn in P2CH:
                        out = tps[p0:p0 + 12, :nn]
                        for g in range(g0, g1):
                            nc.tensor.matmul(
                                out,
                                vo[:, g - g0, 0, :, :].rearrange("p b j -> p (b j)"),
                                phiR[:, g, c0:c0 + nn],
                                start=(g == g0), stop=False,
                                skip_group_check=True)
                            nc.tensor.matmul(
                                out,
                                vo[:, g - g0, 1, :, :].rearrange("p b j -> p (b j)"),
                                phiI[:, g, c0:c0 + nn],
                                start=False, stop=(g == g1 - 1),
                                skip_group_check=True)

                def pass2_reads(b):
                    for p0, c0, nn in P2CH:
                        if b == 0:
                            nc.vector.tensor_copy(tsb[:, c0:c0 + nn],
                                                  tps[p0:p0 + 12, :nn])
                        else:
                            nc.vector.tensor_tensor(
                                tsb[:, c0:c0 + nn], tsb[:, c0:c0 + nn],
                                tps[p0:p0 + 12, :nn], OP.add)

                # ---- main loop over 32 frequency tiles ----
                # plane order (2,0,1,3): the 8th PSUM alloc (bufs=7) aliases
                # plane 2's slot, whose product consumers finish earliest.
                o0 = NRX - CH[1][0]   # y-rows offset within chunk 1
                for ft in range(NFT):
                    et = ep.tile([128, KT, 4, 128], f16, tag="et")
                    sy.dma_start(et[:], Ec.ap()[ft])
                    ps = {}

                    def plane(p):
                        for ci, (c0, nn) in enumerate(CH):
                            ps[(p, ci)] = mps.tile([128, 293], f32, tag="mm",
                                                   name=f"mm{p}{ci}")
                        for kt in range(KT):
                            st = (kt == 0); sp = (kt == KT - 1)
                            for ci, (c0, nn) in enumerate(CH):
                                nc.tensor.matmul(
                                    ps[(p, ci)][:, :nn], et[:, kt, p, :],
                                    xt[:, kt, c0:c0 + nn], start=st, stop=sp)
                        nc.scalar.copy(fy[:, ft, p, :], ps[(p, 1)][:, o0:o0 + 5])

                    # GPSIMD has no PSUM port; DVE TT allows one PSUM operand.
                    # scalar: copy planes 2,3 to SBUF; DVE: the 4 products;
                    # GpSimd: the SBUF-only combines.
                    TTv = nc.vector.tensor_tensor
                    TTg = nc.gpsimd.tensor_tensor
                    plane(2)
                    s2 = [qp.tile([128, 293], f32, tag=f"s2{ci}", name=f"s2{ci}")
                          for ci in range(2)]
                    for ci, (c0, nn) in enumerate(CH):
                        nc.scalar.copy(s2[ci][:, :nn], ps[(2, ci)][:, :nn])
                    plane(0)
                    ta = [qp.tile([128, 293], f32, tag=f"ta{ci}", name=f"ta{ci}")
                          for ci in range(2)]
                    for ci, (c0, nn) in enumerate(CH):      # ta = F1r*F2r
                        TTv(ta[ci][:, :nn], ps[(0, ci)][:, :nn],
                            s2[ci][:, :nn], OP.mult)
                    # pass-2 MMs slot mid-ft; their vt was built a batch ago
                    if ft in (15, 23, 31):
                        bi = {15: 0, 23: 1, 31: 2}[ft]
                        pass2_mms(bi)
                        pass2_reads(bi)
                    plane(1)
                    th = [qp.tile([128, 293], f32, tag=f"th{ci}", name=f"th{ci}")
                          for ci in range(2)]
                    for ci, (c0, nn) in enumerate(CH):      # th = F1i*F2r
                        TTv(th[ci][:, :nn], ps[(1, ci)][:, :nn],
                            s2[ci][:, :nn], OP.mult)
                    plane(3)
                    s3 = [qp.tile([128, 293], f32, tag=f"s3{ci}", name=f"s3{ci}")
                          for ci in range(2)]
                    for ci, (c0, nn) in enumerate(CH):
                        nc.scalar.copy(s3[ci][:, :nn], ps[(3, ci)][:, :nn])
                    TTc = TTv if ft == NFT - 1 else TTg
                    for ci, (c0, nn) in enumerate(CH):
                        sl = slice(c0, c0 + nn)
                        tb = qp.tile([128, 293], f32, tag="tb", name="tb")
                        TTv(tb[:, :nn], ps[(1, ci)][:, :nn],
                            s3[ci][:, :nn], OP.mult)        # tb = F1i*F2i
                        TTc(phiR[:, ft, sl], ta[ci][:, :nn], tb[:, :nn],
                            OP.subtract)
                        tg = qp.tile([128, 293], f32, tag="tg", name="tg")
                        TTv(tg[:, :nn], ps[(0, ci)][:, :nn],
                            s3[ci][:, :nn], OP.mult)        # tg = F1r*F2i
                        TTc(phiI[:, ft, sl], tg[:, :nn], th[ci][:, :nn],
                            OP.add)
                    # vgroups built a full batch before their MMs run
                    if ft in (7, 15, 23, 30):
                        build_v_group({7: 0, 15: 1, 23: 2, 30: 3}[ft])
                    if ft == NFT - 1:
                        build_v_group(4)
                        pass2_mms(3)
                        pass2_reads(3)
                        pass2_mms(4)
                        pass2_reads(4)

            sy.dma_start(tsb_d.ap(), tsb[:])

    nc.compile()
    return nc


def _host_tail(inputs, results):
    """Combine per-core T/nyq into the final [B,1] output on host."""
    w2 = np.asarray(inputs["W_s2"], np.float64)[:, 0]            # [S]
    beta = np.asarray(inputs["b_s2"], np.float64)                # [S]
    wv = np.stack([w2 * w2, w2 * beta, beta * beta], 0) / D      # [3, S]
    W_out = np.asarray(inputs["W_out"], np.float64)              # [1, S]
    b_out = np.asarray(inputs["b_out"], np.float64)              # [1]
    out = np.empty((B, 1), np.float64)
    for core in range(NCORES):
        T = np.asarray(results[core]["tsb_out"], np.float64).reshape(12, NR)
        ny = np.asarray(results[core]["nyq_out"], np.float64).reshape(2, NR)
        F1n, F2n = ny[0], ny[1]
        Q1n, Q2n = F1n[NR - 1], F2n[NR - 1]
        for b in range(BC):
            Tb = T[b * 3:(b + 1) * 3, b * S:(b + 1) * S].copy()  # [3, S]
            pxn = F1n[b * S:(b + 1) * S] * F2n[b * S:(b + 1) * S]
            T1n, T2n = F1n[NRX + b], F2n[NRX + b]
            Tb[0] += pxn * (T1n * T2n)
            Tb[1] += pxn * (T1n * Q2n + T2n * Q1n)
            Tb[2] += pxn * (Q1n * Q2n)
            ip = wv[0] * Tb[0] + wv[1] * Tb[1] + wv[2] * Tb[2]   # [S]
            bp = np.sign(ip) * np.sqrt(np.abs(ip) + 1e-5)
            nrm = max(np.linalg.norm(bp), 1e-12)
            bp = bp / nrm
            out[core * BC + b, 0] = bp @ W_out[0] + b_out[0]
    return out.astype(np.float32)


def kernel(**inputs) -> np.ndarray:
    global _PROGRAM
    if _PROGRAM is None:
        _PROGRAM = _build_program()
    nc = _PROGRAM

    consts = _host_constants(
        inputs["h1"], inputs["h2"], inputs["s1"], inputs["s2"])
    in_maps = [_host_inputs_for_core(c, inputs, *consts)
               for c in range(NCORES)]

    from concourse.bass_utils import run_bass_kernel_spmd
    res = run_bass_kernel_spmd(nc, in_maps, list(range(NCORES)))
    return _host_tail(inputs, res.results)


# revision 21
# speedup vs baseline: 1.0768x; 1.0004x over previous
"""Trainium2 Bass kernel for nn_CompactBilinearPoolingTSP.

Count-sketch + FFT circular convolution collapses (Parseval) into dense
half-spectrum DFT matmuls: F[r,k] = sum_c X[r,c] E[c,k] with E a host
constant, Phi = F1*F2, ip[r] = (1/D) sum_k gamma_k Re(Phi conj(F1y F2y)).
The y-side (sensor branch) is rank-1 in s, so its spectra reduce to
per-b vectors; appended t rows + a ones row ride the same matmuls.

v2 layout: xt is fully host-built ([128, kt, rows] f16, t rows + ones
appended); the main loop computes the 4 DFT planes per 128-freq tile
into PSUM and forms Phi = F1*F2 directly from PSUM (R chain on DVE,
I chain on GpSimd) — no intermediate SBUF spectra.  Pass 2 (contraction
of Phi against the 3 per-b y-vectors over k) is interleaved per 8-tile
batch.  The Nyquist bin k=4096 is handled exactly by a 2-column matmul.
Device emits T[12,585] + Nyquist spectra; signed-sqrt/normalize/W_out
tail runs on host.  Sharding: pure data parallel, 4 batches/core.
"""

import numpy as np

try:
    import concourse.bass  # noqa: F401
except ImportError:  # pragma: no cover
    import sys
    for _p in ("/opt/trn_rl_repo", "/root/.axon_site/_ro/trn_rl_repo"):
        if _p not in sys.path:
            sys.path.append(_p)

_PROGRAM = None

B, S, C, D, SN = 32, 145, 768, 8192, 64
NCORES = 8
BC = B // NCORES          # batches per core = 4
NRX = BC * S              # x rows per core = 580
NR = NRX + BC + 1         # + t rows + ones row = 585
KF = D // 2 + 1           # 4097 distinct freqs
NFT = 32                  # freq tiles of 128 -> 4096; k=4096 handled exactly
KP = NFT * 128
KT = C // 128             # 6 contraction tiles
CH = [(0, 293), (293, 292)]  # row chunks for matmul N (fit one PSUM bank)
NWARM = 8                 # PE warm-up matmuls during DMA head


def _host_constants(h1, h2, s1, s2):
    """E matrices, gamma, V3, Nyquist columns — from hash/sign vectors."""
    h1 = h1.astype(np.int64); h2 = h2.astype(np.int64)
    s1f = s1.astype(np.float64); s2f = s2.astype(np.float64)
    k = np.arange(KP)
    ang1 = (-2.0 * np.pi / D) * (h1[:, None] * k[None, :])
    ang2 = (-2.0 * np.pi / D) * (h2[:, None] * k[None, :])
    E1 = s1f[:, None] * np.exp(1j * ang1)
    E2 = s2f[:, None] * np.exp(1j * ang2)
    # planes: 0=E1r 1=E1i 2=E2r 3=E2i ; layout [NFT, 128k, KT, plane, 128f]
    E = np.stack([E1.real, E1.imag, E2.real, E2.imag], axis=0)  # [4, C, KP]
    E = E.reshape(4, KT, 128, NFT, 128)                          # [p, kt, k, ft, f]
    E = E.transpose(3, 2, 1, 0, 4)                               # [ft, k, kt, p, f]
    E = np.ascontiguousarray(E, dtype=np.float16)

    gamma = np.full(KP, 2.0)
    gamma[0] = 1.0
    gamma_sb = gamma.reshape(NFT, 128).T.astype(np.float32)      # [128, NFT]

    # V3 = gamma * (Q1*Q2) (ones-row spectra product), exact on host
    Q1 = np.ones(C) @ E1
    Q2 = np.ones(C) @ E2
    W3 = Q1 * Q2
    v3 = np.stack([(gamma * W3.real), (gamma * W3.imag)], axis=-1)  # [KP, 2]
    v3_sb = v3.reshape(NFT, 128, 2).transpose(1, 0, 2)              # [128, NFT, 2]
    v3_sb = np.ascontiguousarray(v3_sb, dtype=np.float16)

    # Nyquist k=4096 columns: e[c] = s[c] * (-1)^{h[c]}, per hash
    en = np.stack([s1f * np.where(h1 % 2 == 0, 1.0, -1.0),
                   s2f * np.where(h2 % 2 == 0, 1.0, -1.0)], axis=-1)  # [C, 2]
    en_sb = np.ascontiguousarray(
        en.reshape(KT, 128, 2).transpose(1, 0, 2), dtype=np.float16)  # [128,KT,2]
    return E, gamma_sb, v3_sb, en_sb


def _host_inputs_for_core(core, inputs, E, gamma_sb, v3_sb, en_sb):
    """Per-core in_map (numpy) keyed by dram tensor names."""
    img = np.asarray(inputs["image_embeds"], np.float32)
    sensor = np.asarray(inputs["sensor"], np.float32)
    b0 = core * BC
    rows = np.empty((NR, C), np.float32)
    rows[:NRX] = (img[b0:b0 + BC]
                  + np.asarray(inputs["tok_emb"], np.float32)[1][None, None, :]
                  ).reshape(NRX, C)
    # sensor branch t rows: [BC, C]
    t = (sensor[b0:b0 + BC, 0, :] @ np.asarray(inputs["W_sensor"], np.float32).T
         + np.asarray(inputs["b_sensor"], np.float32)[None, :])
    rows[NRX:NRX + BC] = t
    rows[NR - 1] = 1.0
    # xt layout: [128 part, KT, NR] f16  (channel c = kt*128 + p)
    xtc = np.ascontiguousarray(
        rows.T.reshape(KT, 128, NR).transpose(1, 0, 2), dtype=np.float16)
    return {
        "xtc": xtc,
        "Econst": E,
        "gammac": gamma_sb,
        "v3c": v3_sb,
        "enc": en_sb,
        "warmc": np.zeros((128, 512), np.float16),
    }


def _build_program():
    import concourse.tile as tile
    from concourse import bacc, mybir

    f16 = mybir.dt.float16
    f32 = mybir.dt.float32
    OP = mybir.AluOpType

    nc = bacc.Bacc("TRN2", target_bir_lowering=False, debug=False,
                   num_devices=NCORES)

    xtc = nc.dram_tensor("xtc", [128, KT, NR], f16, kind="ExternalInput")
    Ec = nc.dram_tensor("Econst", [NFT, 128, KT, 4, 128], f16,
                        kind="ExternalInput")
    gammac = nc.dram_tensor("gammac", [128, NFT], f32, kind="ExternalInput")
    v3c = nc.dram_tensor("v3c", [128, NFT, 2], f16, kind="ExternalInput")
    enc = nc.dram_tensor("enc", [128, KT, 2], f16, kind="ExternalInput")
    warmc = nc.dram_tensor("warmc", [128, 512], f16, kind="ExternalInput")
    tsb_d = nc.dram_tensor("tsb_out", [12, NR], f32, kind="ExternalOutput")
    nyq_d = nc.dram_tensor("nyq_out", [2, NR], f32, kind="ExternalOutput")

    with tile.TileContext(nc) as tc:
        with (
            tc.tile_pool(name="const", bufs=1) as cp,
            tc.tile_pool(name="estream", bufs=2) as ep,
            tc.tile_pool(name="vtmp", bufs=2) as vp,
            tc.tile_pool(name="ptmp", bufs=2) as qp,
            tc.tile_pool(name="phip", bufs=1) as pp,
        ):
            # ---- persistent tiles ----
            xt = cp.tile([128, KT, NR], f16)          # rows^T (c on partitions)
            phiR = pp.tile([128, NFT, NR], f16)
            phiI = pp.tile([128, NFT, NR], f16)
            fy = cp.tile([128, NFT, 4, 5], f16)       # spectra of t rows + ones
            vtb = [cp.tile([128, 8, 2, BC, 3], f16, name=f"vtb{i}")
                   for i in range(2)]                 # lhsT for pass 2 (2 bufs)
            gam = cp.tile([128, NFT], f32)
            v3s = cp.tile([128, NFT, 2], f16)
            en = cp.tile([128, KT, 2], f16)
            tsb = cp.tile([12, NR], f32)
            nyqs = cp.tile([2, NR], f32)
            wt = cp.tile([128, 512], f16)
            sy = nc.sync
            # warm tile first; xt split across the scalar and gpsimd DMA
            # queues; E stream rides the sync queue.
            nc.scalar.dma_start(wt[:], warmc.ap())
            nc.scalar.dma_start(xt[:, 0:3, :], xtc.ap()[:, 0:3, :])
            nc.gpsimd.dma_start(xt[:, 3:6, :], xtc.ap()[:, 3:6, :])
            nc.scalar.dma_start(gam[:], gammac.ap())
            nc.scalar.dma_start(v3s[:], v3c.ap())
            nc.scalar.dma_start(en[:], enc.ap())

            # pass-2 batches: vgroup built a full batch ahead (double-
            # buffered vt), MMs slotted mid-ft so nothing waits on DVE
            BATCHES = [(0, 8), (8, 16), (16, 24), (24, 31), (31, 32)]

            def build_v_group(b):
                # j=1 cross terms on DVE, j=0 and the j=2 copy on GpSimd;
                # chains interleaved with separate temps so consecutive ops
                # on each engine don't pay back-to-back semaphore latency.
                g0, g1 = BATCHES[b]
                ng = g1 - g0
                sl = slice(g0, g1)
                vo = vtb[b % 2]
                P1r = fy[:, sl, 0, 0:BC]; P1i = fy[:, sl, 1, 0:BC]
                P2r = fy[:, sl, 2, 0:BC]; P2i = fy[:, sl, 3, 0:BC]
                shp = (128, ng, BC)
                Q1r = fy[:, sl, 0, 4:5].to_broadcast(shp)
                Q1i = fy[:, sl, 1, 4:5].to_broadcast(shp)
                Q2r = fy[:, sl, 2, 4:5].to_broadcast(shp)
                Q2i = fy[:, sl, 3, 4:5].to_broadcast(shp)
                gb = gam[:, sl, None].to_broadcast(shp)

                def tmp(tag):
                    return vp.tile([128, 8, BC], f32, tag=tag,
                                   name=tag)[:, :ng, :]

                va, vb, vc = tmp("va"), tmp("vb"), tmp("vc")
                vd, ve, vf = tmp("vd"), tmp("ve"), tmp("vf")
                wa, wb, wc, wd = tmp("wa"), tmp("wb"), tmp("wc"), tmp("wd")
                TT = nc.vector.tensor_tensor
                TG = nc.gpsimd.tensor_tensor
                # GpSimd: j=0 (gamma * T1T2), R/I interleaved
                TG(wa[:], P1r, P2r, OP.mult)
                TG(wc[:], P1r, P2i, OP.mult)
                TG(wb[:], P1i, P2i, OP.mult)
                TG(wd[:], P1i, P2r, OP.mult)
                TG(wa[:], wa[:], wb[:], OP.subtract)
                TG(wc[:], wc[:], wd[:], OP.add)
                TG(vo[:, :ng, 0, :, 0], wa[:], gb, OP.mult)
                TG(vo[:, :ng, 1, :, 0], wc[:], gb, OP.mult)
                nc.gpsimd.tensor_copy(
                    vo[:, :ng, :, :, 2],
                    v3s[:, sl, :, None].to_broadcast((128, ng, 2, BC)))
                # DVE: j=1 (gamma * (T1Q2 + T2Q1)), R/I interleaved
                TT(va[:], P1r, Q2r, OP.mult)
                TT(vd[:], P1r, Q2i, OP.mult)
                TT(vb[:], P1i, Q2i, OP.mult)
                TT(ve[:], P1i, Q2r, OP.mult)
                TT(va[:], va[:], vb[:], OP.subtract)
                TT(vd[:], vd[:], ve[:], OP.add)
                TT(vb[:], P2r, Q1r, OP.mult)
                TT(ve[:], P2r, Q1i, OP.mult)
                TT(vc[:], P2i, Q1i, OP.mult)
                TT(vf[:], P2i, Q1r, OP.mult)
                TT(vb[:], vb[:], vc[:], OP.subtract)
                TT(ve[:], ve[:], vf[:], OP.add)
                TT(va[:], va[:], vb[:], OP.add)
                TT(vd[:], vd[:], ve[:], OP.add)
                TT(vo[:, :ng, 0, :, 1], va[:], gb, OP.mult)
                TT(vo[:, :ng, 1, :, 1], vd[:], gb, OP.mult)

            with (
                tc.tile_pool(name="mps", bufs=7, space="PSUM") as mps,
                tc.tile_pool(name="p2ps", bufs=1, space="PSUM") as p2,
            ):
                # ---- PE warm-up (HAM ramp) while xt/E stream in ----
                for w in range(NWARM):
                    wps = mps.tile([128, 293], f32, tag="mm", name=f"warm{w}")
                    nc.tensor.matmul(wps[:, :293], wt[:, 0:128], wt[:, 0:293],
                                     start=True, stop=True)
                # ---- Nyquist bin k=4096: F[4096] = sum_c x s (-1)^h ----
                nyp = [mps.tile([128, 293], f32, tag="mm", name=f"nyp{c}")
                       for c in range(2)]
                for ci, (c0, nn) in enumerate(CH):
                    for kt in range(KT):
                        nc.tensor.matmul(
                            nyp[ci][:2, :nn], en[:, kt, :],
                            xt[:, kt, c0:c0 + nn],
                            start=(kt == 0), stop=(kt == KT - 1))
                for ci, (c0, nn) in enumerate(CH):
                    nc.scalar.copy(nyqs[:, c0:c0 + nn], nyp[ci][:2, :nn])
                sy.dma_start(nyq_d.ap(), nyqs[:])

                # chunk0 accumulates at partitions 0:12, chunk1 at 32:44 —
                # disjoint byte ranges, so the two chunks never WAR-stall
                tps = p2.tile([44, 512], f32)
                P2CH = ((0, 0, 512), (32, 512, 73))

                def pass2_mms(b):
                    g0, g1 = BATCHES[b]
                    vo = vtb[b % 2]
                    for p0, c0, nn in P2CH:
                        out = tps[p0:p0 + 12, :nn]
                        for g in range(g0, g1):
                            nc.tensor.matmul(
                                out,
                                vo[:, g - g0, 0, :, :].rearrange("p b j -> p (b j)"),
                                phiR[:, g, c0:c0 + nn],
                                start=(g == g0), stop=False,
                                skip_group_check=True)
                            nc.tensor.matmul(
                                out,
                                vo[:, g - g0, 1, :, :].rearrange("p b j -> p (b j)"),
                                phiI[:, g, c0:c0 + nn],
                                start=False, stop=(g == g1 - 1),
                                skip_group_check=True)

                def pass2_reads(b):
                    for p0, c0, nn in P2CH:
                        if b == 0:
                            nc.vector.tensor_copy(tsb[:, c0:c0 + nn],
                                                  tps[p0:p0 + 12, :nn])
                        else:
                            nc.vector.tensor_tensor(
                                tsb[:, c0:c0 + nn], tsb[:, c0:c0 + nn],
                                tps[p0:p0 + 12, :nn], OP.add)

                # ---- main loop over 32 frequency tiles ----
                # plane order (2,0,1,3): the 8th PSUM alloc (bufs=7) aliases
                # plane 2's slot, whose product consumers finish earliest.
                o0 = NRX - CH[1][0]   # y-rows offset within chunk 1
                for ft in range(NFT):
                    et = ep.tile([128, KT, 4, 128], f16, tag="et")
                    sy.dma_start(et[:], Ec.ap()[ft])
                    ps = {}

                    def plane(p):
                        for ci, (c0, nn) in enumerate(CH):
                            ps[(p, ci)] = mps.tile([128, 293], f32, tag="mm",
                                                   name=f"mm{p}{ci}")
                        for kt in range(KT):
                            st = (kt == 0); sp = (kt == KT - 1)
                            for ci, (c0, nn) in enumerate(CH):
                                nc.tensor.matmul(
                                    ps[(p, ci)][:, :nn], et[:, kt, p, :],
                                    xt[:, kt, c0:c0 + nn], start=st, stop=sp)
                        nc.scalar.copy(fy[:, ft, p, :], ps[(p, 1)][:, o0:o0 + 5])

                    # GPSIMD has no PSUM port; DVE TT allows one PSUM operand.
                    # scalar: copy planes 2,3 to SBUF; DVE: the 4 products;
                    # GpSimd: the SBUF-only combines.
                    TTv = nc.vector.tensor_tensor
                    TTg = nc.gpsimd.tensor_tensor
                    plane(2)
                    s2 = [qp.tile([128, 293], f32, tag=f"s2{ci}", name=f"s2{ci}")
                          for ci in range(2)]
                    for ci, (c0, nn) in enumerate(CH):
                        nc.scalar.copy(s2[ci][:, :nn], ps[(2, ci)][:, :nn])
                    plane(0)
                    ta = [qp.tile([128, 293], f32, tag=f"ta{ci}", name=f"ta{ci}")
                          for ci in range(2)]
                    for ci, (c0, nn) in enumerate(CH):      # ta = F1r*F2r
                        TTv(ta[ci][:, :nn], ps[(0, ci)][:, :nn],
                            s2[ci][:, :nn], OP.mult)
                    # pass-2 MMs slot mid-ft; their vt was built a batch ago
                    if ft in (15, 23, 31):
                        bi = {15: 0, 23: 1, 31: 2}[ft]
                        pass2_mms(bi)
                        pass2_reads(bi)
                    plane(1)
                    th = [qp.tile([128, 293], f32, tag=f"th{ci}", name=f"th{ci}")
                          for ci in range(2)]
                    for ci, (c0, nn) in enumerate(CH):      # th = F1i*F2r
                        TTv(th[ci][:, :nn], ps[(1, ci)][:, :nn],
                            s2[ci][:, :nn], OP.mult)
                    plane(3)
                    s3 = [qp.tile([128, 293], f32, tag=f"s3{ci}", name=f"s3{ci}")
                          for ci in range(2)]
                    for ci, (c0, nn) in enumerate(CH):
                        nc.scalar.copy(s3[ci][:, :nn], ps[(3, ci)][:, :nn])
                    TTc = TTv if ft == NFT - 1 else TTg
                    for ci, (c0, nn) in enumerate(CH):
                        sl = slice(c0, c0 + nn)
                        tb = qp.tile([128, 293], f32, tag="tb", name="tb")
                        TTv(tb[:, :nn], ps[(1, ci)][:, :nn],
                            s3[ci][:, :nn], OP.mult)        # tb = F1i*F2i
                        TTc(phiR[:, ft, sl], ta[ci][:, :nn], tb[:, :nn],
                            OP.subtract)
                        tg = qp.tile([128, 293], f32, tag="tg", name="tg")
                        TTv(tg[:, :nn], ps[(0, ci)][:, :nn],
                            s3[ci][:, :nn], OP.mult)        # tg = F1r*F2i
                        TTc(phiI[:, ft, sl], tg[:, :nn], th[ci][:, :nn],
                            OP.add)
                    # vgroups built a full batch before their MMs run
                    if ft in (7, 15, 23, 30):
                        build_v_group({7: 0, 15: 1, 23: 2, 30: 3}[ft])
                    if ft == NFT - 1:
                        build_v_group(4)
                        pass2_mms(3)
                        pass2_reads(3)
                        pass2_mms(4)
                        pass2_reads(4)

            sy.dma_start(tsb_d.ap(), tsb[:])

    nc.compile()
    return nc


def _host_tail(inputs, results):
    """Combine per-core T/nyq into the final [B,1] output on host."""
    w2 = np.asarray(inputs["W_s2"], np.float64)[:, 0]            # [S]
    beta = np.asarray(inputs["b_s2"], np.float64)                # [S]
    wv = np.stack([w2 * w2, w2 * beta, beta * beta], 0) / D      # [3, S]
    W_out = np.asarray(inputs["W_out"], np.float64)              # [1, S]
    b_out = np.asarray(inputs["b_out"], np.float64)              # [1]
    out = np.empty((B, 1), np.float64)
    for core in range(NCORES):
        T = np.asarray(results[core]["tsb_out"], np.float64).reshape(12, NR)
        ny = np.asarray(results[core]["nyq_out"], np.float64).reshape(2, NR)
        F1n, F2n = ny[0], ny[1]
        Q1n, Q2n = F1n[NR - 1], F2n[NR - 1]
        for b in range(BC):
            Tb = T[b * 3:(b + 1) * 3, b * S:(b + 1) * S].copy()  # [3, S]
            pxn = F1n[b * S:(b + 1) * S] * F2n[b * S:(b + 1) * S]
            T1n, T2n = F1n[NRX + b], F2n[NRX + b]
            Tb[0] += pxn * (T1n * T2n)
            Tb[1] += pxn * (T1n * Q2n + T2n * Q1n)
            Tb[2] += pxn * (Q1n * Q2n)
            ip = wv[0] * Tb[0] + wv[1] * Tb[1] + wv[2] * Tb[2]   # [S]
            bp = np.sign(ip) * np.sqrt(np.abs(ip) + 1e-5)
            nrm = max(np.linalg.norm(bp), 1e-12)
            bp = bp / nrm
            out[core * BC + b, 0] = bp @ W_out[0] + b_out[0]
    return out.astype(np.float32)


def kernel(**inputs) -> np.ndarray:
    global _PROGRAM
    if _PROGRAM is None:
        _PROGRAM = _build_program()
    nc = _PROGRAM

    consts = _host_constants(
        inputs["h1"], inputs["h2"], inputs["s1"], inputs["s2"])
    in_maps = [_host_inputs_for_core(c, inputs, *consts)
               for c in range(NCORES)]

    from concourse.bass_utils import run_bass_kernel_spmd
    res = run_bass_kernel_spmd(nc, in_maps, list(range(NCORES)))
    return _host_tail(inputs, res.results)
